# revision 1
# baseline (speedup 1.0000x reference)
"""Trainium2 Bass kernel for nn_EnhancedTransformerLayer (RoPE attention + MoE).

Sharding: 8 cores; core c -> batch b=c//4, qc=c%4. Four distinct NEFFs (one per
qc), each run on 2 cores (b=0,1). Core qc owns interleaved query blocks
{qc, qc+4, qc+8, qc+12} (4 x 128 tokens) so causal work is balanced, and only
computes K/V up to its last block.

Layout: fused rope->K projection through SBUF (no DRAM round trip for roped
x), V projection in the same span loop writing bf16 V to DRAM scratch, causal
attention with hh-paired exp (one ACT instruction per kc covers both head
halves), bf16 softmax weights, softmax denominator fused into the ctx matmul
via a ones-column appended to V (row 64 of the PSUM accumulator), K bias
dropped (softmax is invariant to it), gating/top-2 done in [8,512]
expert-major layout with gpsimd partition_all_reduce (no transposes), and MoE
with per-expert pre-scaled bf16 inputs so the expert combine and the expert
bias (one [NE,128]-stationary matmul per oc) accumulate in PSUM.
Weight prefetch overlaps earlier phases via pool-stack ordering (wk + span0
pools entered before the Q-phase pool); N=128 fp32r score matmuls padded
to N=256 (4 cy/row -> 1 cy/row; the pad region is never read). ~551us on HW
vs 982us baseline.
"""
import sys, os
sys.path.insert(0, '/opt/trn_rl_repo')
import numpy as np
import ml_dtypes

import concourse.bass as bass
from concourse import bacc
import concourse.tile as tile
from concourse import mybir
from concourse import bass_isa

R = mybir.dt.float32r
F = mybir.dt.float32
BF = mybir.dt.bfloat16
P = 128
B, S, E, H, D, NE = 2, 2048, 1024, 16, 64, 8
NC = E // P
QL = 512
EXP_SCALE = 1.0 / (D ** 0.5)
LN_EPS = 1e-5

_cache = {}


def _build(qc):
    nc = bacc.Bacc("TRN2", target_bir_lowering=False, debug=False, num_devices=8,
                   name=f"moe_qc{qc}", enable_partition_id=False)
    kv_tok = 128 * (qc + 13)
    KCN = kv_tok // P
    # half-spans of 256 tokens (last may be 128)
    spans = []
    s0 = 0
    while s0 < kv_tok:
        sl = min(256, kv_tok - s0)
        spans.append((s0, sl))
        s0 += sl

    def din(name, shape, dt=R):
        return nc.dram_tensor(name, shape, dt, kind="ExternalInput")

    xt = din("xt", [E, S])
    xtq = din("xtq", [E, QL])
    xres = din("xres", [E, QL])
    wq = din("wq", [E, E]); wk = din("wk", [E, E]); wv = din("wv", [E, E])
    bq = din("bq", [P, NC], F)
    bvr = din("bvr", [1, E])
    wo = din("wo", [E, E], BF); bo = din("bo", [P, NC], F)
    gw = din("gw", [E, NE]); gb = din("gb", [NE, 1], F)
    cos2 = din("cos2", [P, S], F); sin2 = din("sin2", [P, S], F)
    cos2q = din("cos2q", [P, QL], F); sin2q = din("sin2q", [P, QL], F)
    trid = din("trid", [P, P], BF)      # tri[k, q] = 1 if q >= k (within a block)
    ew = din("ew", [NE, NC, P, E], BF)
    ebt = din("ebt", [NE, E])
    sels = din("sels", [NE, NE, P])     # sels[k, e, m] = (k == e): row-select stationary
    ln1w = din("ln1w", [P, NC], F); ln1b = din("ln1b", [P, NC], F)
    ln2w = din("ln2w", [P, NC], F); ln2b = din("ln2b", [P, NC], F)
    out = nc.dram_tensor("out", [E, QL], R, kind="ExternalOutput")
    vsch = [nc.dram_tensor(f"vsc{i}", [kv_tok, 512], BF) for i in range(2)]
    ctxd = nc.dram_tensor("ctxd", [E, QL], BF)
    DBG = bool(int(os.environ.get("KERNEL_DEBUG", "0")))
    dbg = {}
    if DBG:
        for nm, shp in (("dbg_h1", [E, QL]), ("dbg_x1", [E, QL]),
                        ("dbg_wfin", [NE, QL]), ("dbg_h2", [E, QL]),
                        ("dbg_den", [2 * NC, QL]), ("dbg_rden", [2 * NC, QL]),
                        ("dbg_qt", [E, QL]), ("dbg_kt", [E, kv_tok])):
            dbg[nm] = nc.dram_tensor(nm, shp, R, kind="ExternalOutput")

    AX = mybir.AxisListType.X
    OP = mybir.AluOpType
    AF = mybir.ActivationFunctionType
    import contextlib

    xt_r = xt.rearrange("(c p) s -> p c s", p=P)

    def rope6(dst, src, cos_sb, sin_sb, tmppool, width, col0=0):
        """dst[:, c] = src[:, c]*cos - src[:, c+4]*sin; dst[:, c+4] = ... + ...
        All reads happen before writes, so dst may alias src (in-place)."""
        sl = slice(col0, col0 + width)
        for c in range(4):
            t1 = tmppool.tile([P, width], F, tag="ropet1")
            t2 = tmppool.tile([P, width], F, tag="ropet2")
            t3 = tmppool.tile([P, width], F, tag="ropet3")
            t4 = tmppool.tile([P, width], F, tag="ropet4")
            nc.vector.tensor_tensor(out=t1[:], in0=src[:, c, sl], in1=cos_sb[:, sl], op=OP.mult)
            nc.vector.tensor_tensor(out=t3[:], in0=src[:, c, sl], in1=sin_sb[:, sl], op=OP.mult)
            nc.vector.tensor_tensor(out=t2[:], in0=src[:, c + 4, sl], in1=sin_sb[:, sl], op=OP.mult)
            nc.vector.tensor_tensor(out=t4[:], in0=src[:, c + 4, sl], in1=cos_sb[:, sl], op=OP.mult)
            nc.vector.tensor_tensor(out=dst[:, c, sl], in0=t1[:], in1=t2[:], op=OP.subtract)
            nc.vector.tensor_tensor(out=dst[:, c + 4, sl], in0=t4[:], in1=t3[:], op=OP.add)

    with tile.TileContext(nc) as tc, \
         nc.allow_low_precision(reason="float32r is bit-identical to float32"), \
         contextlib.ExitStack() as es:
        consts = es.enter_context(tc.tile_pool(name="consts", bufs=1))

        ones_f = consts.tile([P, 1], F, tag="ones_f")
        nc.vector.memset(ones_f[:], 1.0)
        ones = consts.tile([P, 1], R, tag="ones")
        nc.vector.tensor_copy(out=ones[:], in_=ones_f[:])
        ones1_f = consts.tile([1, P], F, tag="ones1_f")
        nc.vector.memset(ones1_f[:], 1.0)
        ones1 = consts.tile([1, P], R, tag="ones1")
        nc.vector.tensor_copy(out=ones1[:], in_=ones1_f[:])
        eps1 = consts.tile([1, 1], F, tag="eps1")
        nc.vector.memset(eps1[:], LN_EPS)
        tri_sb = consts.tile([P, P], BF, tag="tri")
        nc.sync.dma_start(tri_sb[:], trid[:])

        bq_sb = consts.tile([P, NC], F, tag="bq"); nc.sync.dma_start(bq_sb[:], bq[:])
        bo_sb = consts.tile([P, NC], F, tag="bo"); nc.sync.dma_start(bo_sb[:], bo[:])
        gb_sb = consts.tile([NE, 1], F, tag="gb"); nc.sync.dma_start(gb_sb[:], gb[:])
        gw_sb = consts.tile([P, NC, NE], R, tag="gw")
        nc.sync.dma_start(gw_sb[:], gw.rearrange("(c p) g -> p c g", p=P))
        ln_sb = {}
        for nm, t in (("ln1w", ln1w), ("ln1b", ln1b), ("ln2w", ln2w), ("ln2b", ln2b)):
            ln_sb[nm] = consts.tile([P, NC], F, tag=nm, name=nm)
            nc.sync.dma_start(ln_sb[nm][:], t[:])

        attn_cm = tc.tile_pool(name="attn_res", bufs=1)
        attn_res = attn_cm.__enter__()
        qT = attn_res.tile([P, NC, QL], R, tag="qT")
        kT = attn_res.tile([P, NC, kv_tok], R, tag="kT")

        # ===== Phase Q: rope q-chunk + Q projection (wk + span0 prefetch overlap) =====
        wkp_cm = tc.tile_pool(name="wkp", bufs=1)
        wkp = wkp_cm.__enter__()
        wk_sb = wkp.tile([P, NC, E], R, tag="wk_sb")
        kvs_cm = tc.tile_pool(name="kvs", bufs=2)
        kvs = kvs_cm.__enter__()
        with tc.tile_pool(name="qph", bufs=1) as qph, \
             tc.tile_pool(name="qtmp", bufs=1) as qtmp, \
             tc.tile_pool(name="qps_p", bufs=4, space="PSUM") as qps_p:
            cosq_sb = qph.tile([P, QL], F, tag="cosq")
            sinq_sb = qph.tile([P, QL], F, tag="sinq")
            nc.sync.dma_start(cosq_sb[:], cos2q[:])
            nc.sync.dma_start(sinq_sb[:], sin2q[:])
            xtq_sb = qph.tile([P, NC, QL], R, tag="xtq")
            nc.sync.dma_start(xtq_sb[:], xtq.rearrange("(c p) q -> p c q", p=P))
            wq_sb = qph.tile([P, NC, E], R, tag="wq_sb")
            for c in range(NC):
                nc.sync.dma_start(wq_sb[:, c, :],
                                  wq.rearrange("(c p) m -> p c m", p=P)[:, c, :])
            rope6(xtq_sb, xtq_sb, cosq_sb, sinq_sb, qtmp, QL)
            rope_order = [0, 4, 1, 5, 2, 6, 3, 7]
            for oc in range(NC):
                qp = qps_p.tile([P, QL], F, tag="qps")
                for di, dc in enumerate(rope_order):
                    nc.tensor.matmul(
                        qp[:], wq_sb[:, dc, oc * P:(oc + 1) * P], xtq_sb[:, dc, :],
                        start=(di == 0), stop=(di == NC - 1))
                nc.scalar.activation(out=qT[:, oc, :], in_=qp[:],
                                     func=AF.Identity, bias=bq_sb[:, oc:oc + 1])
            for c in range(NC):
                nc.sync.dma_start(wk_sb[:, c, :],
                                  wk.rearrange("(c p) m -> p c m", p=P)[:, c, :])

        # ===== Phase KV: fused rope -> K proj, V proj per half-span =====
        with tc.tile_pool(name="kvw", bufs=1) as kvw, \
             tc.tile_pool(name="kvtmp", bufs=2) as kvtmp, \
             tc.tile_pool(name="vev_p", bufs=3) as vev_p, \
             tc.tile_pool(name="kps_p", bufs=4, space="PSUM") as kps_p, \
             tc.tile_pool(name="vps_p", bufs=3, space="PSUM") as vps_p:
            wv_sb = kvw.tile([P, NC, E], R, tag="wv_sb")
            bv_sb = kvw.tile([1, E], R, tag="bv")
            nc.sync.dma_start(bv_sb[:], bvr[:])
            for c in range(NC):
                nc.sync.dma_start(wv_sb[:, c, :],
                                  wv.rearrange("(c p) m -> p c m", p=P)[:, c, :])
            for (h0, hl) in spans:
                xsp = kvs.tile([P, NC, 256], R, tag="xsp")
                nc.sync.dma_start(xsp[:, :, :hl], xt_r[:, :, h0:h0 + hl])
                chs = kvs.tile([P, 256], F, tag="chs", bufs=1)
                shs = kvs.tile([P, 256], F, tag="shs", bufs=1)
                nc.sync.dma_start(chs[:, :hl], cos2[:, h0:h0 + hl])
                nc.sync.dma_start(shs[:, :hl], sin2[:, h0:h0 + hl])
                rsp = kvs.tile([P, NC, 256], R, tag="rsp")
                rope6(rsp, xsp, chs, shs, kvtmp, hl)
                # K projection (no bias: softmax is invariant to the K bias)
                for oc in range(NC):
                    kp = kps_p.tile([P, 256], F, tag="kps")
                    for dc in range(NC):
                        nc.tensor.matmul(
                            kp[:, :hl], wk_sb[:, dc, oc * P:(oc + 1) * P],
                            rsp[:, dc, :hl], start=(dc == 0), stop=(dc == NC - 1))
                    nc.scalar.copy(out=kT[:, oc, h0:h0 + hl], in_=kp[:, :hl])
                # V projection from the same (un-roped) x span
                for tc_ in range(hl // P):
                    t0 = h0 + tc_ * P
                    for dvs in range(2):
                        vp = vps_p.tile([P, 512], F, tag="vps")
                        for dc in range(NC):
                            nc.tensor.matmul(
                                vp[:], xsp[:, dc, tc_ * P:(tc_ + 1) * P],
                                wv_sb[:, dc, dvs * 512:(dvs + 1) * 512],
                                start=(dc == 0), stop=False)
                        nc.tensor.matmul(
                            vp[:], ones1[:, :], bv_sb[:, dvs * 512:(dvs + 1) * 512],
                            start=False, stop=True)
                        vev = vev_p.tile([P, 512], BF, tag="vev")
                        nc.scalar.copy(out=vev[:], in_=vp[:])
                        nc.sync.dma_start(vsch[dvs][t0:t0 + P, :], vev[:])
        kvs_cm.__exit__(None, None, None)
        wkp_cm.__exit__(None, None, None)

        if DBG:
            for c in range(NC):
                nc.sync.dma_start(dbg["dbg_qt"].rearrange("(c p) q -> p c q", p=P)[:, c, :], qT[:, c, :])
                nc.sync.dma_start(dbg["dbg_kt"].rearrange("(c p) s -> p c s", p=P)[:, c, :], kT[:, c, :])

        # ===== Phase B: causal attention, hh-paired exp, fused denominator =====
        with tc.tile_pool(name="vt_p", bufs=2) as vt_p, \
             tc.tile_pool(name="ctxo_p", bufs=3) as ctxo_p, \
             tc.tile_pool(name="st_p", bufs=3) as st_p, \
             tc.tile_pool(name="rd_p", bufs=2) as rd_p, \
             tc.tile_pool(name="rb_p", bufs=2) as rb_p, \
             tc.tile_pool(name="scps", bufs=2, space="PSUM") as scps, \
             tc.tile_pool(name="ctxps", bufs=2, space="PSUM") as ctxps:
            for hp in range(NC):
                vt = vt_p.tile([P, KCN, 2, 65], BF, tag="vt")
                for hh in range(2):
                    nc.sync.dma_start(
                        vt[:, :, hh, 0:64],
                        vsch[hp // 4].rearrange("(k p) d -> p k d", p=P)
                        [:, :, (hp % 4) * P + hh * 64:(hp % 4) * P + (hh + 1) * 64])
                nc.vector.memset(vt[:, :, :, 64:65], 1.0)
                ctxp2 = [ctxps.tile([65, QL], F, tag=f"ctx{hh}", name=f"ctxp_{hp}_{hh}")
                         for hh in range(2)]
                prev = None
                for kc in range(KCN):
                    j0 = max(0, -(-(kc - qc) // 4))
                    q0 = j0 * P
                    # fp32r matmuls need out free >= 256 for 1 cy/row; widen the
                    # matmul region (exp/ctx still only read the valid [q0:] part)
                    q0m = min(q0, QL - 256)
                    scp = scps.tile([P, 2, QL], F, tag="scp")
                    for hh in range(2):
                        nc.tensor.matmul(
                            scp[:, hh, q0m:], kT[hh * 64:(hh + 1) * 64, hp, kc * P:(kc + 1) * P],
                            qT[hh * 64:(hh + 1) * 64, hp, q0m:], start=True, stop=True)
                    if prev is not None:
                        pkc, pq0, pst = prev
                        for hh in range(2):
                            nc.tensor.matmul(ctxp2[hh][:, pq0:], vt[:, pkc, hh, :],
                                             pst[:, hh, pq0:],
                                             start=(pkc == 0), stop=False)
                    st = st_p.tile([P, 2, QL], BF, tag="st")
                    nc.scalar.activation(out=st[:, :, q0:], in_=scp[:, :, q0:],
                                         func=AF.Exp, scale=EXP_SCALE)
                    if kc >= qc and (kc - qc) % 4 == 0:
                        j = (kc - qc) // 4
                        for hh in range(2):
                            nc.vector.tensor_tensor(
                                out=st[:, hh, j * P:(j + 1) * P],
                                in0=st[:, hh, j * P:(j + 1) * P],
                                in1=tri_sb[:], op=OP.mult)
                    prev = (kc, q0, st)
                pkc, pq0, pst = prev
                for hh in range(2):
                    nc.tensor.matmul(ctxp2[hh][:, pq0:], vt[:, pkc, hh, :],
                                     pst[:, hh, pq0:], start=(pkc == 0), stop=True)
                for hh in range(2):
                    dsb = rd_p.tile([1, QL], F, tag="dsb")
                    nc.vector.tensor_copy(out=dsb[:], in_=ctxp2[hh][64:65, :])
                    rden = rd_p.tile([1, QL], F, tag="rden")
                    nc.vector.reciprocal_approx_fast(out=rden[:], in_=dsb[:])
                    if DBG:
                        denr = rd_p.tile([1, QL], R, tag="denr")
                        nc.vector.tensor_copy(out=denr[:], in_=ctxp2[hh][64:65, :])
                        nc.sync.dma_start(dbg["dbg_den"][hp * 2 + hh:hp * 2 + hh + 1, :], denr[:])
                        rdr = rd_p.tile([1, QL], R, tag="rdr")
                        nc.vector.tensor_copy(out=rdr[:], in_=rden[:])
                        nc.sync.dma_start(dbg["dbg_rden"][hp * 2 + hh:hp * 2 + hh + 1, :], rdr[:])
                    rb = rb_p.tile([64, QL], F, tag="rb")
                    nc.gpsimd.partition_broadcast(rb[:], rden[:])
                    ctxo = ctxo_p.tile([64, QL], BF, tag="ctxo")
                    nc.vector.tensor_tensor(
                        out=ctxo[:], in0=ctxp2[hh][0:64, :], in1=rb[:], op=OP.mult)
                    nc.sync.dma_start(
                        ctxd[hp * P + hh * 64:hp * P + (hh + 1) * 64, :], ctxo[:])
        attn_cm.__exit__(None, None, None)

        # ===== LN helper =====
        def layernorm(src, dst, wtile, btile, tmp, ps_row):
            sp_ = ps_row.tile([1, QL], F, tag="lnrow")
            for c in range(NC):
                nc.tensor.matmul(sp_[:], ones[:], src[:, c, :],
                                 start=(c == 0), stop=(c == NC - 1))
            s2p = ps_row.tile([1, QL], F, tag="lnrow2")
            for c in range(NC):
                sq = tmp.tile([P, QL], R, tag="lnsq", bufs=2)
                nc.scalar.activation(out=sq[:], in_=src[:, c, :], func=AF.Square)
                nc.tensor.matmul(s2p[:], ones[:], sq[:],
                                 start=(c == 0), stop=(c == NC - 1))
            mean = tmp.tile([1, QL], F, tag="lnmean")
            nc.scalar.mul(out=mean[:], in_=sp_[:], mul=1.0 / E)
            msq = tmp.tile([1, QL], R, tag="lnmsq")
            nc.vector.tensor_tensor(out=msq[:], in0=mean[:], in1=mean[:], op=OP.mult)
            var = tmp.tile([1, QL], R, tag="lnvar")
            nc.vector.scalar_tensor_tensor(out=var[:], in0=s2p[:], scalar=1.0 / E,
                                           in1=msq[:], op0=OP.mult, op1=OP.subtract)
            std = tmp.tile([1, QL], F, tag="lnstd")
            nc.scalar.activation(out=std[:], in_=var[:], func=AF.Sqrt, bias=eps1[:])
            rstd = tmp.tile([1, QL], F, tag="lnrstd")
            nc.vector.reciprocal_approx_fast(out=rstd[:], in_=std[:])
            mb = tmp.tile([P, QL], F, tag="lnmb")
            nc.gpsimd.partition_broadcast(mb[:], mean[:])
            rbb = tmp.tile([P, QL], F, tag="lnrb")
            nc.gpsimd.partition_broadcast(rbb[:], rstd[:])
            for c in range(NC):
                t = tmp.tile([P, QL], R, tag="lnt", bufs=2)
                nc.vector.tensor_tensor(out=t[:], in0=src[:, c, :], in1=mb[:], op=OP.subtract)
                nc.vector.tensor_tensor(out=t[:], in0=t[:], in1=rbb[:], op=OP.mult)
                nc.vector.tensor_scalar(out=dst[:, c, :], in0=t[:],
                                        scalar1=wtile[:, c:c + 1], scalar2=btile[:, c:c + 1],
                                        op0=OP.mult, op1=OP.add)

        # ===== Phase C: out-proj + LN1 (in place: h1 -> x1) + gating =====
        cres = es.enter_context(tc.tile_pool(name="cres", bufs=1))
        x1 = cres.tile([P, NC, QL], R, tag="x1")   # holds h1, then LN1 output
        wbc = cres.tile([P, NE, QL], R, tag="wbc")
        wfin = cres.tile([NE, QL], R, tag="wfin")
        ebt_sb = cres.tile([NE, E], R, tag="ebt")
        nc.sync.dma_start(ebt_sb[:], ebt[:])
        sels_sb = cres.tile([NE, NE, P], R, tag="sels")
        nc.sync.dma_start(sels_sb[:], sels[:])
        ewp_cm = tc.tile_pool(name="ewp", bufs=2)
        ewp = ewp_cm.__enter__()
        with tc.tile_pool(name="ch1", bufs=1) as ch1, \
             tc.tile_pool(name="cslab", bufs=4) as cslab:
            with tc.tile_pool(name="wop", bufs=1) as wop, \
                 tc.tile_pool(name="cps8", bufs=1, space="PSUM") as cps8:
                wo_sb = wop.tile([P, NC, E], BF, tag="wo_sb")
                for c in range(NC):
                    nc.sync.dma_start(wo_sb[:, c, :],
                                      wo.rearrange("(c p) m -> p c m", p=P)[:, c, :])
                xres_sb = ch1.tile([P, NC, QL], R, tag="xres")
                nc.sync.dma_start(xres_sb[:], xres.rearrange("(c p) q -> p c q", p=P))
                aps = [cps8.tile([P, QL], F, tag=f"ap{oc}", name=f"ap{oc}")
                       for oc in range(NC)]
                for dc in range(NC):
                    ctxc = cslab.tile([P, QL], BF, tag="ctxc")
                    nc.sync.dma_start(
                        ctxc[:], ctxd.rearrange("(c p) q -> p c q", p=P)[:, dc, :])
                    for oc in range(NC):
                        nc.tensor.matmul(aps[oc][:], wo_sb[:, dc, oc * P:(oc + 1) * P],
                                         ctxc[:], start=(dc == 0), stop=(dc == NC - 1))
                for oc in range(NC):
                    nc.vector.scalar_tensor_tensor(
                        out=x1[:, oc, :], in0=aps[oc][:], scalar=bo_sb[:, oc:oc + 1],
                        in1=xres_sb[:, oc, :], op0=OP.add, op1=OP.add)
            with tc.tile_pool(name="ct", bufs=1) as ct, \
                 tc.tile_pool(name="cps", bufs=2, space="PSUM") as cps, \
                 tc.tile_pool(name="crow", bufs=2, space="PSUM") as crow:
                if DBG:
                    for c in range(NC):
                        nc.sync.dma_start(dbg["dbg_h1"].rearrange("(c p) q -> p c q", p=P)[:, c, :], x1[:, c, :])
                layernorm(x1, x1, ln_sb["ln1w"], ln_sb["ln1b"], ct, crow)
                if DBG:
                    for c in range(NC):
                        nc.sync.dma_start(dbg["dbg_x1"].rearrange("(c p) q -> p c q", p=P)[:, c, :], x1[:, c, :])

                # gating in [NE, QL] layout
                gp = crow.tile([NE, QL], F, tag="gps")
                for c in range(NC):
                    nc.tensor.matmul(gp[:], gw_sb[:, c, :], x1[:, c, :],
                                     start=(c == 0), stop=(c == NC - 1))
                gexp = ct.tile([NE, QL], R, tag="gexp")
                nc.scalar.activation(out=gexp[:], in_=gp[:], func=AF.Exp, bias=gb_sb[:])
                den_all = ct.tile([NE, QL], F, tag="den_all")
                nc.gpsimd.partition_all_reduce(den_all[:], gexp[:], channels=NE,
                                               reduce_op=bass_isa.ReduceOp.add)
                rgb = ct.tile([NE, QL], F, tag="rgb")
                nc.vector.reciprocal_approx_fast(out=rgb[:], in_=den_all[:])

                m1b = ct.tile([NE, QL], R, tag="m1b")
                nc.gpsimd.partition_all_reduce(m1b[:], gexp[:], channels=NE,
                                               reduce_op=bass_isa.ReduceOp.max)
                msel = ct.tile([NE, QL], R, tag="msel")
                nc.vector.tensor_tensor(out=msel[:], in0=gexp[:], in1=m1b[:], op=OP.is_equal)
                inv = ct.tile([NE, QL], R, tag="inv")
                nc.vector.tensor_scalar(out=inv[:], in0=msel[:], scalar1=-1.0, scalar2=1.0,
                                        op0=OP.mult, op1=OP.add)
                g2 = ct.tile([NE, QL], R, tag="g2")
                nc.vector.tensor_tensor(out=g2[:], in0=gexp[:], in1=inv[:], op=OP.mult)
                m2b = ct.tile([NE, QL], R, tag="m2b")
                nc.gpsimd.partition_all_reduce(m2b[:], g2[:], channels=NE,
                                               reduce_op=bass_isa.ReduceOp.max)
                msel2 = ct.tile([NE, QL], R, tag="msel2")
                nc.vector.tensor_tensor(out=msel2[:], in0=g2[:], in1=m2b[:], op=OP.is_equal)
                nc.vector.tensor_tensor(out=msel[:], in0=msel[:], in1=msel2[:], op=OP.add)
                wsel = ct.tile([NE, QL], R, tag="wsel")
                nc.vector.tensor_tensor(out=wsel[:], in0=gexp[:], in1=msel[:], op=OP.mult)
                nc.vector.tensor_tensor(out=wfin[:], in0=wsel[:], in1=rgb[:], op=OP.mult)
                if DBG:
                    nc.sync.dma_start(dbg["dbg_wfin"][:], wfin[:])
                for e in range(NE):
                    wbp = cps.tile([P, QL], F, tag="cbig")
                    nc.tensor.matmul(wbp[:], sels_sb[:, e, :], wfin[:], start=True, stop=True)
                    nc.vector.tensor_copy(out=wbc[:, e, :], in_=wbp[:])

        # ===== Phase D: MoE, experts accumulate in PSUM =====
        h2 = cres.tile([P, NC, QL], R, tag="h2")
        with tc.tile_pool(name="x1e_p", bufs=2) as x1e_p, \
             tc.tile_pool(name="dps", bufs=1, space="PSUM") as dps:
            yps = [dps.tile([P, QL], F, tag=f"yp{oc}", name=f"yp{oc}") for oc in range(NC)]
            for oc in range(NC):
                nc.tensor.matmul(yps[oc][:], ebt_sb[:, oc * P:(oc + 1) * P], wfin[:],
                                 start=True, stop=False)
            for e in range(NE):
                ew_sl = ewp.tile([P, NC, E], BF, tag="ew_sl")
                nc.sync.dma_start(ew_sl[:], ew[e].rearrange("c p d -> p c d"))
                x1e = x1e_p.tile([P, NC, QL], BF, tag="x1e")
                for dc in range(NC):
                    nc.vector.tensor_tensor(out=x1e[:, dc, :], in0=x1[:, dc, :],
                                            in1=wbc[:, e, :], op=OP.mult)
                for oc in range(NC):
                    for dc in range(NC):
                        nc.tensor.matmul(
                            yps[oc][:], ew_sl[:, dc, oc * P:(oc + 1) * P], x1e[:, dc, :],
                            start=False, stop=(e == NE - 1 and dc == NC - 1))
            for oc in range(NC):
                nc.vector.tensor_tensor(out=h2[:, oc, :], in0=yps[oc][:],
                                        in1=x1[:, oc, :], op=OP.add)
            if DBG:
                for c in range(NC):
                    nc.sync.dma_start(dbg["dbg_h2"].rearrange("(c p) q -> p c q", p=P)[:, c, :], h2[:, c, :])
        ewp_cm.__exit__(None, None, None)

        # ===== Phase E: LN2 + store =====
        with tc.tile_pool(name="et", bufs=1) as et, \
             tc.tile_pool(name="erow", bufs=2, space="PSUM") as erow:
            layernorm(h2, h2, ln_sb["ln2w"], ln_sb["ln2b"], et, erow)
            for c in range(NC):
                nc.sync.dma_start(out.rearrange("(c p) q -> p c q", p=P)[:, c, :], h2[:, c, :])

    nc.compile()
    return nc


def _prep_inputs(inputs):
    x = np.asarray(inputs['x'], dtype=np.float32)
    ipw = np.asarray(inputs['in_proj_w'], dtype=np.float32)
    ipb = np.asarray(inputs['in_proj_b'], dtype=np.float32)
    opw = np.asarray(inputs['out_proj_w'], dtype=np.float32)
    opb = np.asarray(inputs['out_proj_b'], dtype=np.float32)
    gww = np.asarray(inputs['gate_w'], dtype=np.float32)
    gbb = np.asarray(inputs['gate_b'], dtype=np.float32)
    eww = np.asarray(inputs['expert_w'], dtype=np.float32)
    ebb = np.asarray(inputs['expert_b'], dtype=np.float32)

    perm = np.empty(E, dtype=np.int64)
    idx = 0
    for h in range(H):
        for i in range(D // 2):
            perm[idx] = 64 * h + 2 * i; idx += 1
    for h in range(H):
        for i in range(D // 2):
            perm[idx] = 64 * h + 2 * i + 1; idx += 1

    Wq, Wk, Wv = ipw[0:E], ipw[E:2 * E], ipw[2 * E:3 * E]
    bq_, bk_, bv_ = ipb[0:E], ipb[E:2 * E], ipb[2 * E:3 * E]
    common = {
        "wq": np.ascontiguousarray(Wq[:, perm].T),
        "wk": np.ascontiguousarray(Wk[:, perm].T),
        "wv": np.ascontiguousarray(Wv[:, perm].T),
        "bq": np.ascontiguousarray(bq_.reshape(NC, P).T),
        "bvr": bv_.reshape(1, E).copy(),
        "wo": np.ascontiguousarray(opw.T).astype(ml_dtypes.bfloat16),
        "bo": np.ascontiguousarray(opb.reshape(NC, P).T),
        "gw": np.ascontiguousarray(gww.T),
        "gb": gbb.reshape(NE, 1).copy(),
        "ew": np.ascontiguousarray(
            eww.transpose(0, 2, 1).reshape(NE, NC, P, E)).astype(ml_dtypes.bfloat16),
        "ebt": np.ascontiguousarray(ebb),
        "ln1w": np.ascontiguousarray(np.asarray(inputs['ln1_w'], np.float32).reshape(NC, P).T),
        "ln1b": np.ascontiguousarray(np.asarray(inputs['ln1_b'], np.float32).reshape(NC, P).T),
        "ln2w": np.ascontiguousarray(np.asarray(inputs['ln2_w'], np.float32).reshape(NC, P).T),
        "ln2b": np.ascontiguousarray(np.asarray(inputs['ln2_b'], np.float32).reshape(NC, P).T),
        "trid": np.ascontiguousarray(
            (np.arange(P)[None, :] >= np.arange(P)[:, None])).astype(ml_dtypes.bfloat16),
        "sels": np.ascontiguousarray(
            np.repeat(np.eye(NE, dtype=np.float32)[:, :, None], P, axis=2)),
    }
    inv_freq = 1.0 / (10000.0 ** (np.arange(0, D, 2, dtype=np.float64) / D))
    freqs = np.arange(S, dtype=np.float64)[:, None] * inv_freq[None, :]
    cos_t = np.cos(freqs).T.astype(np.float32)
    sin_t = np.sin(freqs).T.astype(np.float32)
    cos2 = np.ascontiguousarray(np.tile(cos_t, (4, 1)))
    sin2 = np.ascontiguousarray(np.tile(sin_t, (4, 1)))
    common["cos2"] = cos2
    common["sin2"] = sin2

    in_maps = []
    for c in range(8):
        b, qc = c // 4, c % 4
        blocks = [qc + 4 * i for i in range(4)]
        cols = np.concatenate([np.arange(blk * P, (blk + 1) * P) for blk in blocks])
        xtb = np.ascontiguousarray(x[b].T)
        xtp = np.ascontiguousarray(xtb[perm])
        m = dict(common)
        m["xt"] = xtp
        m["xtq"] = np.ascontiguousarray(xtp[:, cols])
        m["xres"] = np.ascontiguousarray(xtb[:, cols])
        m["cos2q"] = np.ascontiguousarray(cos2[:, cols])
        m["sin2q"] = np.ascontiguousarray(sin2[:, cols])
        in_maps.append(m)
    return in_maps


def _run_multi(ncs, in_maps):
    """Run the 4 NEFFs concurrently: graph qc on devices {qc, qc+4} (b=0,1)."""
    import jax
    from jax.sharding import Mesh, PartitionSpec
    from jax.experimental.shard_map import shard_map
    from concourse import bass2jax
    from concourse import mybir as _mb

    bass2jax.install_neuronx_cc_hook()
    devices = jax.devices()

    if "jits" not in _cache:
        _cache["jits"] = {}
    handles = []
    for qc in range(4):
        nc = ncs[qc]
        if qc not in _cache["jits"]:
            in_names, out_names, out_avals, zero_outs = [], [], [], []
            for alloc in nc.m.functions[0].allocations:
                if not isinstance(alloc, _mb.MemoryLocationSet):
                    continue
                name = alloc.memorylocations[0].name
                if alloc.kind == "ExternalInput":
                    in_names.append(name)
                elif alloc.kind == "ExternalOutput":
                    out_names.append(name)
                    shape = tuple(alloc.tensor_shape)
                    dtype = _mb.dt.np(alloc.dtype)
                    out_avals.append(jax.core.ShapedArray(shape, dtype))
                    zero_outs.append(np.zeros(shape, dtype))
            n_params = len(in_names)
            all_names = in_names + out_names
            donate = tuple(range(n_params, n_params + len(out_names)))

            def _body(*args, _nc=nc, _avals=tuple(out_avals), _all=tuple(all_names),
                      _outs=tuple(out_names)):
                outs = bass2jax._bass_exec_p.bind(
                    *args, out_avals=_avals, in_names=_all, out_names=_outs,
                    lowering_input_output_aliases=(),
                    sim_require_finite=True, sim_require_nnan=True, nc=_nc)
                return tuple(outs)

            devs = [devices[qc], devices[qc + 4]]
            mesh = Mesh(np.asarray(devs), ("core",))
            nio = n_params + len(zero_outs)
            sharded = jax.jit(
                shard_map(_body, mesh=mesh,
                          in_specs=(PartitionSpec("core"),) * nio,
                          out_specs=(PartitionSpec("core"),) * len(out_names),
                          check_rep=False),
                donate_argnums=donate, keep_unused=True)
            _cache["jits"][qc] = (sharded, in_names, out_names, zero_outs)
        sharded, in_names, out_names, zero_outs = _cache["jits"][qc]
        per_core = [[np.asarray(in_maps[b * 4 + qc][n]) for n in in_names] for b in range(2)]
        concat_in = [np.concatenate([per_core[b][i] for b in range(2)], axis=0)
                     for i in range(len(in_names))]
        concat_zero = [np.concatenate([z, z], axis=0) for z in zero_outs]
        handles.append((sharded, concat_in, concat_zero, out_names))

    outs = []
    for sharded, concat_in, concat_zero, out_names in handles:
        outs.append((sharded(*concat_in, *concat_zero), out_names))
    results = [None] * 8
    for qc, (arrs, out_names) in enumerate(outs):
        arrs = [np.asarray(a) for a in arrs]
        for b in range(2):
            rm = {}
            for i, n in enumerate(out_names):
                full = arrs[i]
                half = full.shape[0] // 2
                rm[n] = full[b * half:(b + 1) * half]
            results[b * 4 + qc] = rm
    return results


def _ensure_ntff_hook():
    import types
    try:
        from antenv.axon_hooks import get_axon_ntff_profile_hook  # noqa
        return True
    except ImportError:
        pass
    try:
        import antenv
        sys.path.insert(0, '/root/.axon_site')
        from trn_agent_boot.trn_boot import _ntff_profile_via_ctypes
        hook = _ntff_profile_via_ctypes('/opt/axon/libaxon_pjrt.so')
        if hook is None:
            return False
        mod = types.ModuleType('antenv.axon_hooks')
        _state = {'hook': hook}
        mod.set_axon_ntff_profile_hook = lambda h: _state.__setitem__('hook', h)
        mod.get_axon_ntff_profile_hook = lambda: _state['hook']
        sys.modules['antenv.axon_hooks'] = mod
        antenv.axon_hooks = mod
        return True
    except Exception as e:
        print(f"ntff hook setup failed: {e}")
        return False


def kernel(**inputs):
    if "ncs" not in _cache:
        _cache["ncs"] = [_build(qc) for qc in range(4)]
    ncs = _cache["ncs"]
    in_maps = _prep_inputs(inputs)

    trace = bool(int(os.environ.get("KERNEL_TRACE", "0")))
    if trace and _ensure_ntff_hook():
        import tempfile
        from antenv.axon_hooks import get_axon_ntff_profile_hook
        hook = get_axon_ntff_profile_hook()
        tmpdir = tempfile.mkdtemp()
        _run_multi(ncs, in_maps)  # warm-up/compile outside the profile window
        with hook(tmpdir, list(range(8))):
            results = _run_multi(ncs, in_maps)
        _cache["ntff_dir"] = tmpdir
        print(f"ntff dir: {tmpdir}")
    else:
        results = _run_multi(ncs, in_maps)
    _cache["last_results"] = results

    out = np.empty((B, S, E), dtype=np.float32)
    for c in range(8):
        b, qc = c // 4, c % 4
        o = results[c]["out"]  # [E, QL]
        for i in range(4):
            blk = qc + 4 * i
            out[b, blk * P:(blk + 1) * P, :] = o[:, i * P:(i + 1) * P].T
    return out

